# revision 17
# baseline (speedup 1.0000x reference)
"""FP8Linear (dequant matmul + bias) Trainium2 Bass kernel.

out[b,s,n] = x[b,s,:] @ (qweight[n,:] * repeat(scales[n,:], 128)).T + bias[n]

Full shapes: x [4,2048,4096] f32, qweight [16384,4096] f32,
scales [16384,32] f32, bias [16384] f32 -> out [4,2048,16384] f32.

Sharding: tensor-parallel column split over 8 cores. Core c owns
out_features rows [c*2048, (c+1)*2048) of qweight/scales/bias; x is
replicated; per-core outputs [8192, 2048] are concatenated on the host.

Per-core design (Tile framework, fully static/unrolled):
  - Weights: load qweight shard [o,k] f32, dequantize+cast to bf16 on DVE
    (group scales broadcast along the free dim), then one XBAR SBUF->SBUF
    transpose per 128-row o-tile producing the SBUF-resident
    wT [128(kp), 32(kt), 2048(o)] bf16 (128KB/partition), k = kt*128+kp.
  - x: load 128-row s-tiles f32, cast to bf16 on the scalar engine, one
    XBAR transpose -> xT [128(kp), 32(kt), 128(s)].
  - matmul: psum[s,o-slice] += xT[:,kt,:].T @ wT[:,kt,o-slice], accumulated
    over 32 k-tiles into 4 PSUM banks (o = 4*512); 8 banks = 2 s-tiles in
    flight. Redundant Ldweights (4 matmuls share each stationary tile) are
    stripped post-compile (walrus runs with --enable-ldw-opt=false).
  - bias is broadcast across partitions once via a rank-1 bf16 matmul and
    fused into the PSUM->SBUF eviction (DVE tensor_tensor add).
  - plain copies ride the scalar-engine HWDGE ring, XBAR transposes the SP
    ring, so the DMA queues never pay copy<->transpose xbar-mode switches.
"""

import numpy as np

IN_F = 4096
OUT_F = 16384
N_CORES = 8
S_TOTAL = 4 * 2048  # 8192
P = 128
NB_FREE = 512  # matmul moving free dim == one PSUM bank of f32

_nc_cache = {}


def _build_bass(s_total, in_f, o_shard, main_repeat=1, dedup_ldw=True,
                null_kernel=False, ring_split=True):
    import concourse.bacc as bacc
    import concourse.mybir as mybir
    import concourse.tile as tile

    f32 = mybir.dt.float32
    bf16 = mybir.dt.bfloat16
    ADD = mybir.AluOpType.add

    kt_n = in_f // P           # 32 k-tiles (== scale groups)
    ot_n = o_shard // P        # 16 o-tiles
    st_n = s_total // P        # 64 s-tiles
    nb_n = o_shard // NB_FREE  # 4 psum banks per s-tile
    half = in_f // 2

    nc = bacc.Bacc("TRN2", target_bir_lowering=False, debug=False,
                   num_devices=N_CORES)
    x = nc.dram_tensor("x", [s_total, in_f], f32, kind="ExternalInput")
    qw = nc.dram_tensor("qweight", [o_shard, in_f], f32, kind="ExternalInput")
    sc = nc.dram_tensor("scales", [o_shard, kt_n], f32, kind="ExternalInput")
    bias = nc.dram_tensor("bias", [o_shard], f32, kind="ExternalInput")
    out = nc.dram_tensor("out", [s_total, o_shard], f32, kind="ExternalOutput")

    cp = nc.scalar if ring_split else nc.sync  # plain-copy HWDGE ring

    with tile.TileContext(nc) as tc:
        with (
            tc.tile_pool(name="const", bufs=1) as const,
            tc.tile_pool(name="stage", bufs=2) as stage,
            tc.tile_pool(name="psum", bufs=8, space="PSUM") as psum,
        ):
            wT = const.tile([P, kt_n, o_shard], bf16)
            bias_bc = const.tile([P, o_shard], f32)
            sc_all = const.tile([P, ot_n, kt_n], f32)
            ones16 = const.tile([1, P], bf16)

            # ---- bias broadcast across partitions via rank-1 bf16 matmul ----
            # (no SBUF->SBUF DMA copies: those serialize against the XBAR
            # transposes that fill wT/xT)
            bias_row32 = stage.tile([1, o_shard], f32, tag="ld")
            cp.dma_start(out=bias_row32, in_=bias[None, :])
            bias_row16 = stage.tile([1, o_shard], bf16, tag="xT")
            nc.vector.tensor_copy(out=bias_row16, in_=bias_row32)
            nc.vector.memset(ones16, 1.0)
            for nb in range(nb_n):
                pb = psum.tile([P, NB_FREE], f32, tag="acc", name="pb")
                nc.tensor.matmul(
                    pb,
                    lhsT=ones16,
                    rhs=bias_row16[:, nb * NB_FREE:(nb + 1) * NB_FREE],
                    start=True,
                    stop=True,
                )
                nc.vector.tensor_copy(
                    out=bias_bc[:, nb * NB_FREE:(nb + 1) * NB_FREE], in_=pb
                )
            cp.dma_start(
                out=sc_all, in_=sc[:, :].rearrange("(ot p) g -> p ot g", p=P)
            )

            # ---- weights: load f32 -> dequant+cast (DVE) -> XBAR transpose ----
            # Emission is software-pipelined (load ot+1 before dequantizing ot)
            # so the q-load of the next o-tile enters the DMA ring ahead of the
            # transpose that has to wait for this o-tile's dequant.
            if null_kernel:
                cp.dma_start(out=out[0:1, 0:1], in_=qw[0:1, 0:1])
                main_repeat = 0
                ot_n = 0
                st_n = 0

            gch = half // P  # scale groups per half-row chunk

            def emit_w_load(ot):
                halves = []
                for h in range(2):
                    q32 = stage.tile([P, half], f32, tag="ld", name=f"q32_{h}")
                    cp.dma_start(
                        out=q32,
                        in_=qw[ot * P:(ot + 1) * P, h * half:(h + 1) * half],
                    )
                    halves.append(q32)
                return halves

            def emit_w_deq_transpose(ot, halves):
                wq16 = stage.tile([P, in_f], bf16, tag="cast")
                for h, q32 in enumerate(halves):
                    nc.vector.tensor_tensor(
                        wq16[:, h * half:(h + 1) * half].rearrange(
                            "p (g j) -> p g j", j=P
                        ),
                        q32.rearrange("p (g j) -> p g j", j=P),
                        sc_all[:, ot, h * gch:(h + 1) * gch, None].to_broadcast(
                            (P, gch, P)
                        ),
                        mybir.AluOpType.mult,
                    )
                nc.sync.dma_start(
                    out=wT[:, :, ot * P:(ot + 1) * P], in_=wq16, transpose=True
                )

            pending = None
            for ot in range(ot_n):
                q32 = emit_w_load(ot)
                if pending is not None:
                    emit_w_deq_transpose(ot - 1, pending)
                pending = q32
            if pending is not None:
                emit_w_deq_transpose(ot_n - 1, pending)

            # ---- main loop over s-tiles ----
            for st_rep in range(st_n * main_repeat):
                st = st_rep % st_n
                x16 = stage.tile([P, in_f], bf16, tag="cast")
                for h in range(2):
                    x32 = stage.tile([P, half], f32, tag="ld", name=f"x32_{h}")
                    cp.dma_start(
                        out=x32, in_=x[st * P:(st + 1) * P, h * half:(h + 1) * half]
                    )
                    nc.scalar.copy(out=x16[:, h * half:(h + 1) * half], in_=x32)
                xT = stage.tile([P, kt_n, P], bf16, tag="xT")
                nc.sync.dma_start(out=xT, in_=x16, transpose=True)

                accs = [
                    psum.tile([P, NB_FREE], f32, tag="acc", name=f"acc{nb}")
                    for nb in range(nb_n)
                ]
                for kt in range(kt_n):
                    lhsT = xT[:, kt, :]
                    for nb in range(nb_n):
                        nc.tensor.matmul(
                            accs[nb],
                            lhsT=lhsT,
                            rhs=wT[:, kt, nb * NB_FREE:(nb + 1) * NB_FREE],
                            start=(kt == 0),
                            stop=(kt == kt_n - 1),
                        )
                o_sb = stage.tile([P, o_shard], f32, tag="out")
                for nb in range(nb_n):
                    nc.vector.tensor_tensor(
                        o_sb[:, nb * NB_FREE:(nb + 1) * NB_FREE],
                        accs[nb],
                        bias_bc[:, nb * NB_FREE:(nb + 1) * NB_FREE],
                        ADD,
                    )
                cp.dma_start(out=out[st * P:(st + 1) * P, :], in_=o_sb)

    return _finish(nc, dedup_ldw)


def _finish(nc, dedup_ldw):
    nc.compile()
    if dedup_ldw:
        _strip_redundant_ldweights(nc)
    return nc


def _strip_redundant_ldweights(nc):
    """Drop InstLdweights that reload the exact weights already resident.

    nc.tensor.matmul() is self-loading: compile() splits every Matmult into
    Ldweights+Matmult, and with --enable-ldw-opt=false walrus never dedups.
    Our inner loop issues 4 matmuls (one per PSUM bank) off the same
    stationary tile, so 3/4 of the Ldweights are redundant. Only drop a
    Ldweights when (a) it has no sync waits/updates of its own and (b) no
    other PE instruction that could disturb the loaded weights ran since the
    identical previous load.
    """
    import concourse.mybir as mybir

    removed = 0
    for blk in nc.m.functions[0].blocks:
        insts = list(blk.instructions)
        keep = []
        last_key = None
        changed = False
        for inst in insts:
            if isinstance(inst, mybir.InstLdweights):
                si = inst.sync_info
                has_sync = bool(si and (si.on_wait or si.on_update))
                key = (
                    str(inst.ins[0]),
                    str(inst.perf_mode),
                    str(inst.is_transpose),
                    str(inst.tile_position),
                    str(inst.tile_size),
                )
                if not has_sync and key == last_key:
                    removed += 1
                    changed = True
                    continue
                last_key = key
            elif isinstance(inst, mybir.InstMatmult):
                if inst.ldweights is not False:
                    last_key = None  # self-loading matmul changes weights
            elif inst.engine == mybir.EngineType.PE and inst.is_executable():
                last_key = None
            keep.append(inst)
        if changed:
            blk.instructions = keep
    return removed


def _get_nc(key, *args):
    if key not in _nc_cache:
        _nc_cache[key] = _build_bass(*args)
    return _nc_cache[key]


def kernel(x, qweight, scales, bias):
    from concourse.bass_utils import run_bass_kernel_spmd

    o_shard = OUT_F // N_CORES

    xf = np.ascontiguousarray(
        np.asarray(x, dtype=np.float32).reshape(S_TOTAL, IN_F)
    )
    qweight = np.asarray(qweight, dtype=np.float32)
    scales = np.asarray(scales, dtype=np.float32)
    bias = np.asarray(bias, dtype=np.float32)

    nc = _get_nc("full", S_TOTAL, IN_F, o_shard)

    in_maps = []
    for c in range(N_CORES):
        sl = slice(c * o_shard, (c + 1) * o_shard)
        in_maps.append(
            {
                "x": xf,
                "qweight": np.ascontiguousarray(qweight[sl]),
                "scales": np.ascontiguousarray(scales[sl]),
                "bias": np.ascontiguousarray(bias[sl]),
            }
        )

    res = run_bass_kernel_spmd(nc, in_maps, core_ids=list(range(N_CORES)))

    out = np.empty((S_TOTAL, OUT_F), dtype=np.float32)
    for c in range(N_CORES):
        out[:, c * o_shard:(c + 1) * o_shard] = res.results[c]["out"]
    return out.reshape(4, 2048, OUT_F)
